# revision 11
# baseline (speedup 1.0000x reference)
"""Trainium2 Bass kernel for additive (Bahdanau) attention.

reference:
    proj_f = features @ W1_w + W1_b          # [B, L, ATT]
    proj_h = (hidden @ W2_w + W2_b)[:, None] # [B, 1, ATT]
    scores = tanh(proj_f + proj_h) @ V_w + V_b   # [B, L]
    alpha  = softmax(scores, axis=1)
    context = einsum('bl,ble->be', alpha, features)
    returns (alpha, context)

Sharding: data-parallel over batch B=64 across 8 cores (8 examples/core).
Weights replicated. No collectives.

Per-core algorithm (X = 8 examples):
  - cast-DMA features f32 -> bf16 DRAM scratch (gpsimd SWDGE).
  - HW transpose-DMA bf16 DRAM -> SBUF: fT [ENC_chunk=128, L=1024] x8.
  - main matmul in [ATT_part, L_free] orientation: lhsT = W1 chunk
    (natural layout), rhs = fT. PSUM [128, 512] f32.
  - ACT applies tanh fused with per-partition bias = (W1_b + W2_b +
    hidden @ W2_w) transposed - computed in a small f32 prepass.
  - V-dot on PE: scores[1, L] += V_chunk[128,1].T @ tanh_tile, accumulated
    over ATT chunks in PSUM.  (V_b dropped: softmax is shift-invariant.)
  - softmax per example on DVE/ACT (free-dim reduces on [1, 1024]).
  - context on DVE: tensor_tensor_reduce over fT tiles with alpha
    replicated across partitions (gpsimd partition_broadcast).
"""

import numpy as np

B, L, ENC, DEC, ATT = 64, 1024, 1024, 1024, 1024
N_CORES = 8
X = B // N_CORES  # examples per core
P = 128
NE = ENC // P  # 8
NA = ATT // P  # 8
ND = DEC // P  # 8
LH = 512       # free-dim half for fp32 PSUM bank
NL = L // LH   # 2

_CACHE = {}


def _build():
    import concourse.bacc as bacc
    import concourse.mybir as mybir
    import concourse.tile as tile

    f32, bf16 = mybir.dt.float32, mybir.dt.bfloat16
    Tanh = mybir.ActivationFunctionType.Tanh
    Exp = mybir.ActivationFunctionType.Exp
    add = mybir.AluOpType.add
    mult = mybir.AluOpType.mult
    AX = mybir.AxisListType.X

    nc = bacc.Bacc("TRN2", target_bir_lowering=False, debug=False, num_devices=N_CORES)

    feats = nc.declare_dram_parameter("features", [X, L, ENC], f32, isOutput=False)
    hid = nc.declare_dram_parameter("hidden_state", [X, DEC], f32, isOutput=False)
    w1 = nc.declare_dram_parameter("W1_w", [ENC, ATT], f32, isOutput=False)
    w1b = nc.declare_dram_parameter("W1_b", [ATT], f32, isOutput=False)
    w2 = nc.declare_dram_parameter("W2_w", [DEC, ATT], f32, isOutput=False)
    w2b = nc.declare_dram_parameter("W2_b", [ATT], f32, isOutput=False)
    vw = nc.declare_dram_parameter("V_w", [ATT], f32, isOutput=False)
    alpha_o = nc.declare_dram_parameter("alpha", [X, L], f32, isOutput=True)
    ctx_o = nc.declare_dram_parameter("context", [X, ENC], f32, isOutput=True)

    eye_dram = nc.inline_tensor(np.eye(P, dtype=np.float32), "eye128")

    with tile.TileContext(nc) as tc:
        with (
            tc.tile_pool(name="const", bufs=1) as const,
            tc.tile_pool(name="dram", bufs=2, space="DRAM") as dram,
            tc.tile_pool(name="ft", bufs=2 * NE) as ftp,
            tc.tile_pool(name="mm", bufs=3, space="PSUM") as psum,
            tc.tile_pool(name="sc", bufs=2, space="PSUM") as spsum,
            tc.tile_pool(name="ct", bufs=1, space="PSUM") as ctpsum,
            tc.tile_pool(name="tb", bufs=4) as tp,
            tc.tile_pool(name="jk", bufs=2) as jp,
            tc.tile_pool(name="al", bufs=2) as alp,
            tc.tile_pool(name="ms", bufs=1) as ms,
        ):
            # ---------------- prep: constants & weights ----------------
            eye = const.tile([P, P], f32, tag="eye")
            nc.sync.dma_start(eye[:], eye_dram[:, :])

            w1bf = []
            for e in range(NE):
                t = const.tile([P, ATT], bf16, tag=f"w1_{e}")
                nc.gpsimd.dma_start(t[:], w1[P * e : P * (e + 1), :])
                w1bf.append(t)

            w2t = []
            for e in range(ND):
                t = const.tile([P, ATT], f32, tag=f"w2_{e}")
                nc.sync.dma_start(t[:], w2[P * e : P * (e + 1), :])
                w2t.append(t)

            # hT_all[p, c, x] = hid[x, 128c + p]  (fine-grained strided DMA)
            hT = ms.tile([P, ND, X], f32, tag="hT")
            hid_t = hid.rearrange("x (c p) -> p c x", p=P)
            for c in range(ND):
                nc.gpsimd.dma_start(hT[:, c, :], hid_t[:, c, :])

            # bias vectors transposed: bT[p, c] = v[128c + p]
            w1bT = ms.tile([P, NA], f32, tag="w1bT")
            nc.gpsimd.dma_start(w1bT[:], w1b.rearrange("(c p) -> p c", p=P))
            w2bT = ms.tile([P, NA], f32, tag="w2bT")
            nc.gpsimd.dma_start(w2bT[:], w2b.rearrange("(c p) -> p c", p=P))
            vwT = ms.tile([P, NA], f32, tag="vwT")
            nc.gpsimd.dma_start(vwT[:], vw.rearrange("(c p) -> p c", p=P))
            vwbf = ms.tile([P, NA], bf16, tag="vwbf")
            nc.vector.tensor_copy(vwbf[:], vwT[:])

            bT = ms.tile([P, NA], f32, tag="bT")
            nc.vector.tensor_add(bT[:], w1bT[:], w2bT[:])

            # proj_h transposed, plus bias: phb[p, a, x]
            phb = ms.tile([P, NA, X], f32, tag="phb")
            for a in range(NA):
                ph_ps = psum.tile([P, X], f32, tag="mm")
                for e in range(ND):
                    nc.tensor.matmul(
                        ph_ps[:],
                        w2t[e][:, P * a : P * (a + 1)],
                        hT[:, e, :],
                        start=(e == 0),
                        stop=(e == ND - 1),
                    )
                nc.vector.tensor_scalar_add(phb[:, a, :], ph_ps[:], bT[:, a : a + 1])

            # ---------------- outputs accumulated in SBUF ----------------
            ctx_sb = ms.tile([P, NE * X], f32, tag="ctx_sb")

            # ---------------- main per-example pipeline ----------------
            # V-dot matmuls are delayed by one (a, lh) block so the PE never
            # waits on the ACT tanh of the block it just produced.
            pending = []

            def flush_pending():
                for sc_ap, vw_ap, tb_ap, st, sp in pending:
                    nc.tensor.matmul(sc_ap, vw_ap, tb_ap, start=st, stop=sp)
                pending.clear()

            for x in range(X):
                # bf16 cast to DRAM scratch
                fbf = dram.tile([L, ENC], bf16, tag="fbf")
                for c in range(8):
                    nc.gpsimd.dma_start(
                        fbf[P * c : P * (c + 1), :], feats[x, P * c : P * (c + 1), :]
                    )
                # HW transpose-DMA: fT[e] = fbf[:, 128e:128(e+1)].T
                fts = []
                for e in range(NE):
                    ft = ftp.tile([P, L], bf16, tag="ft")
                    nc.sync.dma_start(ft[:], fbf[:, P * e : P * (e + 1)], transpose=True)
                    fts.append(ft)

                sc_ps = spsum.tile([1, L], f32, tag="sc")
                for a in range(NA):
                    for lh in range(NL):
                        pp = psum.tile([P, LH], f32, tag="mm")
                        for e in range(NE):
                            nc.tensor.matmul(
                                pp[:],
                                w1bf[e][:, P * a : P * (a + 1)],
                                fts[e][:, LH * lh : LH * (lh + 1)],
                                start=(e == 0),
                                stop=(e == NE - 1),
                            )
                        flush_pending()
                        tb = tp.tile([P, LH], bf16, tag="tb")
                        nc.scalar.activation(tb[:], pp[:], Tanh, bias=phb[:, a, x : x + 1])
                        pending.append(
                            (
                                sc_ps[:, LH * lh : LH * (lh + 1)],
                                vwbf[:, a : a + 1],
                                tb[:],
                                a == 0,
                                a == NA - 1,
                            )
                        )

                flush_pending()
                # softmax over L on partition 0
                negm = alp.tile([1, 1], f32, tag="negm")
                nc.vector.tensor_reduce(
                    negm[:], sc_ps[:], axis=AX, op=mybir.AluOpType.max, negate=True
                )
                esb = alp.tile([1, L], f32, tag="esb")
                ssum = alp.tile([1, 1], f32, tag="ssum")
                nc.scalar.activation(
                    esb[:], sc_ps[:], Exp, bias=negm[:], accum_out=ssum[:]
                )
                rinv = alp.tile([1, 1], f32, tag="rinv")
                nc.vector.reciprocal(rinv[:], ssum[:])
                a32 = alp.tile([1, L], f32, tag="a32")
                nc.vector.tensor_scalar_mul(a32[:], esb[:], rinv[:])
                nc.sync.dma_start(alpha_o[x, :], a32[:])
                abf = alp.tile([1, L], bf16, tag="abf")
                nc.vector.tensor_scalar_mul(abf[:], esb[:], rinv[:])
                arep = alp.tile([P, L], bf16, tag="arep")
                nc.gpsimd.partition_broadcast(arep[:], abf[:])

                # context: ctx[e-chunk] = sum_l fT[e][:, l] * alpha[l]
                for e in range(NE):
                    jk = jp.tile([P, L], f32, tag="jk")
                    nc.vector.scalar_tensor_tensor(
                        out=jk[:],
                        in0=fts[e][:],
                        scalar=1.0,
                        in1=arep[:],
                        op0=mult,
                        op1=mult,
                        accum_out=ctx_sb[:, X * e + x : X * e + x + 1],
                    )

            # ---------------- epilogue: outputs ----------------
            out_sb = ms.tile([X, ENC], f32, tag="out_sb")
            for e in range(NE):
                ct_ps = ctpsum.tile([X, P], f32, tag="ctps")
                nc.tensor.transpose(ct_ps[:], ctx_sb[:, X * e : X * (e + 1)], eye[:])
                nc.vector.tensor_copy(out_sb[:, P * e : P * (e + 1)], ct_ps[:])
            nc.sync.dma_start(ctx_o[:, :], out_sb[:])

    nc.compile()
    return nc


def kernel(features, hidden_state, W1_w, W1_b, W2_w, W2_b, V_w, V_b):
    from concourse.bass_utils import run_bass_kernel_spmd

    if "nc" not in _CACHE:
        _CACHE["nc"] = _build()
    nc = _CACHE["nc"]

    features = np.ascontiguousarray(np.asarray(features, dtype=np.float32))
    hidden_state = np.ascontiguousarray(np.asarray(hidden_state, dtype=np.float32))
    W1_w = np.ascontiguousarray(np.asarray(W1_w, dtype=np.float32))
    W1_b = np.ascontiguousarray(np.asarray(W1_b, dtype=np.float32))
    W2_w = np.ascontiguousarray(np.asarray(W2_w, dtype=np.float32))
    W2_b = np.ascontiguousarray(np.asarray(W2_b, dtype=np.float32))
    V_w = np.ascontiguousarray(np.asarray(V_w, dtype=np.float32))

    in_maps = []
    for c in range(N_CORES):
        in_maps.append(
            {
                "features": np.ascontiguousarray(features[c * X : (c + 1) * X]),
                "hidden_state": np.ascontiguousarray(hidden_state[c * X : (c + 1) * X]),
                "W1_w": W1_w,
                "W1_b": W1_b,
                "W2_w": W2_w,
                "W2_b": W2_b,
                "V_w": V_w,
            }
        )

    res = run_bass_kernel_spmd(nc, in_maps, list(range(N_CORES)), **_CACHE.get("run_kwargs", {}))
    _CACHE["last_result"] = res
    alpha = np.concatenate([res.results[c]["alpha"] for c in range(N_CORES)], axis=0)
    context = np.concatenate([res.results[c]["context"] for c in range(N_CORES)], axis=0)
    return alpha, context
